# revision 1
# baseline (speedup 1.0000x reference)
"""DigitCapsule dynamic-routing kernel for 8 Trainium2 NeuronCores.

Key restructuring: u_hat (B,R,D,O) = 188 MB is NEVER materialized.
  s[b,(d,o)]  = sum_{(r,i)} (c[r,d]*W[r,d,o,i]) * u[b,r,i]      (matmul over (r,i))
  b_upd[r,d]  = sum_{i,o} W[r,d,o,i] * G[(r,i),(d,o)],
  G[(r,i),(d,o)] = sum_b u[b,(r,i)] * v[b,(d,o)]                 (matmul over b)

Sharding: route nodes R=1152 are split 144/core across 8 cores.  Softmax
(over d) and the b-logit update are then fully local; the only collective
is one 160 KB AllReduce of the partial s per routing iteration (3 total).

Layouts on device (per core, all fp32):
  u_nat [2,128,1152] : u[b,(r,i)] with b = h*128+p
  uT    [128,9,256]  : u[(r,i),b] with (r,i) = t*128+p
  Wp    [128,9,160]  : W[(r,i),(d,o)]  ((r,i)=t*128+p, f=d*16+o)
  Jm    [128,128]    : block-diag ones (16 blocks of 8x8) - sums/broadcasts
                       over the i sub-axis of a partition group via the PE
Softmax is computed in the expanded [(r,i),(d,o)] layout; the denominator
accumulated over all 160 columns is 16x the true one, so the kernel tracks
s' = s/16 and corrects inside squash:  v = s' * 256*sqrt(T')/(1+256*T'),
T' = sum(s'^2).
"""

import numpy as np

import concourse.bass as bass
import concourse.mybir as mybir
import concourse.tile as tile
from concourse.bass_utils import run_bass_kernel_spmd

N_CORES = 8
B, R, D, O, I_CH = 256, 1152, 10, 16, 8
RL = R // N_CORES           # 144 route nodes per core
KRI = RL * I_CH             # 1152 = (r,i) contraction length per core
NT = KRI // 128             # 9 partition tiles of (r,i)
DO = D * O                  # 160
NB = B // 128               # 2 batch halves
N_ITER = 3

f32 = mybir.dt.float32
ALU = mybir.AluOpType
AF = mybir.ActivationFunctionType

_ws_ctr = [0]


def _split_excess_waits(nc, max_waits=1):
    """Walrus in this container only lowers one sync-wait per instruction.
    Hoist excess waits onto NOPs inserted before the instruction on the
    same engine (same-order execution => identical semantics)."""
    n_split = 0
    for f in nc.m.functions:
        for bb in f.blocks:
            out = []
            changed = False
            for ins in bb.instructions:
                si = ins.sync_info
                waits = list(si.on_wait) if (si is not None and si.on_wait) else []
                if len(waits) > max_waits:
                    changed = True
                    n_split += 1
                    head, rest = waits[:-max_waits], waits[-max_waits:]
                    while head:
                        chunk, head = head[:max_waits], head[max_waits:]
                        _ws_ctr[0] += 1
                        nop = mybir.InstNoOp(name=f"I-ws{_ws_ctr[0]}")
                        nop.engine = ins.engine
                        nop.sync_info = mybir.SyncInfo(on_wait=chunk, on_update=[])
                        out.append(nop)
                    ins.sync_info = mybir.SyncInfo(
                        on_wait=rest,
                        on_update=list(si.on_update) if si.on_update else [],
                    )
                out.append(ins)
            if changed:
                bb.instructions = out
    return n_split


def _build_nc():
    nc = bass.Bass(
        "TRN2", target_bir_lowering=False, debug=False, num_devices=N_CORES
    )
    u_nat_d = nc.dram_tensor("u_nat", [NB, 128, KRI], f32, kind="ExternalInput")
    uT_d = nc.dram_tensor("uT", [128, NT, B], f32, kind="ExternalInput")
    Wp_d = nc.dram_tensor("Wp", [128, NT, DO], f32, kind="ExternalInput")
    Jm_d = nc.dram_tensor("Jm", [128, 128], f32, kind="ExternalInput")
    v_out_d = nc.dram_tensor("v_out", [NB, 128, DO], f32, kind="ExternalOutput")

    rg = [list(range(N_CORES))]

    with tile.TileContext(nc) as tc:
        with (
            tc.tile_pool(name="persist", bufs=1) as pp_,
            tc.tile_pool(name="iter", bufs=2) as ip_,
            tc.tile_pool(name="small", bufs=2) as sp_,
            tc.tile_pool(name="dram", bufs=2, space="DRAM") as dp_,
            tc.tile_pool(name="ps_s", bufs=2, space="PSUM") as ps_s,
            tc.tile_pool(name="ps_g", bufs=2, space="PSUM") as ps_g,
            tc.tile_pool(name="ps_bd", bufs=2, space="PSUM") as ps_bd,
            tc.tile_pool(name="ps_t", bufs=1, space="PSUM") as ps_t,
        ):
            # ---- persistent tensors ----
            u_nat = pp_.tile([128, NB, KRI], f32)
            uT = pp_.tile([128, NT, B], f32)
            Wp = pp_.tile([128, NT, DO], f32)
            J = pp_.tile([128, 128], f32)
            ones = pp_.tile([128, 128], f32)
            blog = pp_.tile([128, NT, DO], f32)

            for h in range(NB):
                nc.sync.dma_start(u_nat[:, h, :], u_nat_d[h])
            nc.sync.dma_start(uT[:], uT_d[:])
            nc.sync.dma_start(Wp[:], Wp_d[:])
            nc.sync.dma_start(J[:], Jm_d[:])
            nc.gpsimd.memset(ones[:], 1.0)
            nc.vector.memset(blog[:], 0.0)

            for it in range(N_ITER):
                last = it == N_ITER - 1
                # ---- softmax over d (expanded layout), fold 1/16 ----
                e = ip_.tile([128, NT, DO], f32, name=f"e{it}", tag="e")
                den16 = ip_.tile([128, NT], f32, name=f"den{it}", tag="den")
                for t in range(NT):
                    nc.scalar.activation(
                        e[:, t, :], blog[:, t, :], AF.Exp,
                        accum_out=den16[:, t : t + 1],
                    )
                recip16 = ip_.tile([128, NT], f32, name=f"rc{it}", tag="rc")
                nc.vector.reciprocal(recip16[:], den16[:])
                # CW = (e * recip16) * Wp   == (c/16) * W
                CW = ip_.tile([128, NT, DO], f32, name=f"cw{it}", tag="cw")
                for t in range(NT):
                    nc.vector.scalar_tensor_tensor(
                        CW[:, t, :], e[:, t, :], recip16[:, t : t + 1],
                        Wp[:, t, :], op0=ALU.mult, op1=ALU.mult,
                    )
                # ---- mm1: s'[b,(d,o)] = sum_(r,i) uT.T @ CW ----
                s_sb = ip_.tile([128, NB, DO], f32, name=f"s{it}", tag="s")
                for h in range(NB):
                    s_ps = ps_s.tile([128, DO], f32, name=f"sps{it}_{h}", tag="sps")
                    for t in range(NT):
                        nc.tensor.matmul(
                            s_ps[:],
                            uT[:, t, h * 128 : (h + 1) * 128],
                            CW[:, t, :],
                            start=(t == 0),
                            stop=(t == NT - 1),
                        )
                    nc.vector.tensor_copy(s_sb[:, h, :], s_ps[:])
                # ---- AllReduce partial s over the 8 cores ----
                inb = dp_.tile([NB, 128, DO], f32, name=f"inb{it}", tag="inb")
                outb = dp_.tile([NB, 128, DO], f32, name=f"outb{it}", tag="outb")
                for h in range(NB):
                    nc.sync.dma_start(inb[h], s_sb[:, h, :])
                nc.gpsimd.collective_compute(
                    "AllReduce", ALU.add, replica_groups=rg,
                    ins=[inb.opt()], outs=[outb.opt()],
                )
                sf = ip_.tile([128, NB, DO], f32, name=f"sf{it}", tag="sf")
                for h in range(NB):
                    nc.sync.dma_start(sf[:, h, :], outb[h])
                # ---- squash with global norm: v = s' * 256*sqrt(T')/(1+256*T') ----
                ppart = sp_.tile([128, NB], f32, name=f"pp{it}", tag="pp")
                for h in range(NB):
                    sqscr = sp_.tile([128, DO], f32, name=f"sq{it}_{h}", tag="sq")
                    nc.scalar.activation(
                        sqscr[:], sf[:, h, :], AF.Square,
                        accum_out=ppart[:, h : h + 1],
                    )
                ppsum = sp_.tile([128, 1], f32, name=f"pps{it}", tag="pps")
                nc.vector.reduce_sum(ppsum[:], ppart[:], axis=mybir.AxisListType.X)
                # T' broadcast to every partition via ones-matmul
                T_ps = ps_t.tile([128, 1], f32, name=f"T{it}", tag="T")
                nc.tensor.matmul(T_ps[:], ones[:], ppsum[:], start=True, stop=True)
                q = sp_.tile([128, 1], f32, name=f"q{it}", tag="q")
                nc.vector.tensor_scalar(
                    q[:], T_ps[:], 256.0, 1.0, op0=ALU.mult, op1=ALU.add
                )
                qinv = sp_.tile([128, 1], f32, name=f"qi{it}", tag="qi")
                nc.vector.reciprocal(qinv[:], q[:])
                rt = sp_.tile([128, 1], f32, name=f"rt{it}", tag="rt")
                nc.scalar.activation(rt[:], T_ps[:], AF.Sqrt, scale=65536.0)
                g = sp_.tile([128, 1], f32, name=f"g{it}", tag="g")
                nc.vector.tensor_tensor(g[:], rt[:], qinv[:], op=ALU.mult)
                v_sb = ip_.tile([128, NB, DO], f32, name=f"v{it}", tag="v")
                for h in range(NB):
                    nc.vector.tensor_scalar_mul(
                        v_sb[:, h, :], sf[:, h, :], g[:, 0:1]
                    )
                if last:
                    for h in range(NB):
                        nc.sync.dma_start(v_out_d[h], v_sb[:, h, :])
                else:
                    # ---- mm2: G = u_nat.T @ v ; Hred[r,i;d] = sum_o Wp*G ----
                    Hred = ip_.tile([128, NT, D], f32, name=f"hr{it}", tag="hr")
                    for t in range(NT):
                        G_ps = ps_g.tile([128, DO], f32, name=f"G{it}_{t}", tag="G")
                        for h in range(NB):
                            nc.tensor.matmul(
                                G_ps[:],
                                u_nat[:, h, t * 128 : (t + 1) * 128],
                                v_sb[:, h, :],
                                start=(h == 0),
                                stop=(h == NB - 1),
                            )
                        Ht = sp_.tile([128, DO], f32, name=f"ht{it}_{t}", tag="ht")
                        nc.vector.tensor_tensor(
                            Ht[:], Wp[:, t, :], G_ps[:], op=ALU.mult
                        )
                        nc.vector.reduce_sum(
                            Hred[:, t, :],
                            Ht[:].rearrange("p (d o) -> p d o", d=D, o=O),
                            axis=mybir.AxisListType.X,
                        )
                    # ---- i-sum + broadcast via J; blog += bd ----
                    for t in range(NT):
                        bd_ps = ps_bd.tile([128, D], f32, name=f"bd{it}_{t}", tag="bd")
                        nc.tensor.matmul(
                            bd_ps[:], J[:], Hred[:, t, :], start=True, stop=True
                        )
                        blog_v = blog[:, t, :].rearrange("p (d o) -> p d o", d=D, o=O)
                        nc.vector.tensor_tensor(
                            blog_v,
                            blog_v,
                            bd_ps[:].unsqueeze(2).broadcast_to([128, D, O]),
                            op=ALU.add,
                        )

    _split_excess_waits(nc, 1)
    return nc


_NC_CACHE = {}


def _get_nc():
    if "nc" not in _NC_CACHE:
        _NC_CACHE["nc"] = _build_nc()
    return _NC_CACHE["nc"]


def _prep_core_inputs(u, W, c):
    r0, r1 = c * RL, (c + 1) * RL
    u2 = np.ascontiguousarray(u[:, r0:r1, :]).reshape(B, KRI)
    u_nat = np.ascontiguousarray(u2.reshape(NB, 128, KRI))
    uT = np.ascontiguousarray(
        np.ascontiguousarray(u2.T).reshape(NT, 128, B).transpose(1, 0, 2)
    )
    Wp2 = np.ascontiguousarray(W[0, r0:r1].transpose(0, 3, 1, 2)).reshape(KRI, DO)
    Wp = np.ascontiguousarray(Wp2.reshape(NT, 128, DO).transpose(1, 0, 2))
    return {"u_nat": u_nat, "uT": uT, "Wp": Wp}


def kernel(u, W, _trace=False):
    u = np.asarray(u, dtype=np.float32)
    W = np.asarray(W, dtype=np.float32)
    assert u.shape == (B, R, I_CH) and W.shape == (1, R, D, O, I_CH)
    Jm = np.kron(np.eye(16, dtype=np.float32), np.ones((8, 8), np.float32))
    in_maps = []
    for c in range(N_CORES):
        m = _prep_core_inputs(u, W, c)
        m["Jm"] = Jm
        in_maps.append(m)
    nc = _get_nc()
    res = run_bass_kernel_spmd(
        nc, in_maps, core_ids=list(range(N_CORES)), trace=_trace
    )
    v = res.results[0]["v_out"].reshape(B, D, O).astype(np.float32)
    if _trace:
        return v, res
    return v


# revision 23
# speedup vs baseline: 6958.8231x; 6958.8231x over previous
"""DigitCapsule dynamic-routing kernel for 8 Trainium2 NeuronCores.

Key restructuring: u_hat (B,R,D,O) = 188 MB is NEVER materialized.
  s[b,(d,o)]  = sum_{(r,i)} (c[r,d]*W[r,d,o,i]) * u[b,r,i]      (matmul over (r,i))
  b_upd[r,d]  = sum_{i,o} W[r,d,o,i] * G[(r,i),(d,o)],
  G[(r,i),(d,o)] = sum_b u[b,(r,i)] * v[b,(d,o)]                 (matmul over b)

Sharding: route nodes R=1152 are split 144/core across 8 cores.  Softmax
(over d) and the b-logit update are then fully local; the only collective
is one 160 KB AllReduce of the partial s per routing iteration (3 total).

Layouts on device (per core, all fp32):
  u_nat [2,128,1152] : u[b,(r,i)] with b = h*128+p
  uT    [128,9,256]  : u[(r,i),b] with (r,i) = t*128+p
  Wp    [128,9,160]  : W[(r,i),(d,o)]  ((r,i)=t*128+p, f=d*16+o)
  Jm    [128,128]    : block-diag ones (16 blocks of 8x8) - sums/broadcasts
                       over the i sub-axis of a partition group via the PE
Softmax is computed in the expanded [(r,i),(d,o)] layout; the denominator
accumulated over all 160 columns is 16x the true one, so the kernel tracks
s' = s/16 and corrects inside squash:  v = s' * 256*sqrt(T')/(1+256*T'),
T' = sum(s'^2).
"""

import numpy as np

import concourse.bass as bass
import concourse.mybir as mybir
import concourse.tile as tile
from concourse.bass_utils import run_bass_kernel_spmd
from concourse.tile import add_dep_helper

N_CORES = 8
B, R, D, O, I_CH = 256, 1152, 10, 16, 8
RL = R // N_CORES           # 144 route nodes per core
KRI = RL * I_CH             # 1152 = (r,i) contraction length per core
NT = KRI // 128             # 9 partition tiles of (r,i)
DO = D * O                  # 160
NB = B // 128               # 2 batch halves
N_ITER = 3

f32 = mybir.dt.float32
ALU = mybir.AluOpType
AF = mybir.ActivationFunctionType

_ws_ctr = [0]


def _split_excess_waits(nc, max_waits=1):
    """Walrus in this container only lowers one sync-wait per instruction.
    Hoist excess waits onto NOPs inserted before the instruction on the
    same engine (same-order execution => identical semantics)."""
    n_split = 0
    for f in nc.m.functions:
        for bb in f.blocks:
            out = []
            changed = False
            for ins in bb.instructions:
                si = ins.sync_info
                waits = list(si.on_wait) if (si is not None and si.on_wait) else []
                if len(waits) > max_waits:
                    changed = True
                    n_split += 1
                    head, rest = waits[:-max_waits], waits[-max_waits:]
                    while head:
                        chunk, head = head[:max_waits], head[max_waits:]
                        _ws_ctr[0] += 1
                        nop = mybir.InstNoOp(name=f"I-ws{_ws_ctr[0]}")
                        nop.engine = ins.engine
                        nop.sync_info = mybir.SyncInfo(on_wait=chunk, on_update=[])
                        out.append(nop)
                    ins.sync_info = mybir.SyncInfo(
                        on_wait=rest,
                        on_update=list(si.on_update) if si.on_update else [],
                    )
                out.append(ins)
            if changed:
                bb.instructions = out
    return n_split


def _build_nc(reps=1, warm_mms=50, prewarm=27):
    nc = bass.Bass(
        "TRN2", target_bir_lowering=False, debug=False, num_devices=N_CORES
    )
    u_nat_d = nc.dram_tensor("u_nat", [NB, 128, KRI], f32, kind="ExternalInput")
    uT_d = nc.dram_tensor("uT", [128, NT, B], f32, kind="ExternalInput")
    Wp_d = nc.dram_tensor("Wp", [128, NT, DO], f32, kind="ExternalInput")
    Jm_d = nc.dram_tensor("Jm", [128, 128], f32, kind="ExternalInput")
    v_out_d = nc.dram_tensor("v_out", [NB, 128, DO], f32, kind="ExternalOutput")

    rg = [list(range(N_CORES))]

    with tile.TileContext(nc) as tc:
        with (
            tc.tile_pool(name="persist", bufs=1) as pp_,
            tc.tile_pool(name="iter", bufs=2) as ip_,
            tc.tile_pool(name="small", bufs=2) as sp_,
            tc.tile_pool(name="dram", bufs=2, space="DRAM") as dp_,
            tc.tile_pool(name="ps_s", bufs=1, space="PSUM") as ps_s,
            tc.tile_pool(name="ps_g", bufs=2, space="PSUM") as ps_g,
            tc.tile_pool(name="ps_bd", bufs=2, space="PSUM") as ps_bd,
            tc.tile_pool(name="ps_t", bufs=1, space="PSUM") as ps_t,
        ):
            # ---- persistent tensors ----
            u_nat = pp_.tile([128, NB, KRI], f32)
            uT = pp_.tile([128, NT, B], f32)
            Wp = pp_.tile([128, NT, DO], f32)
            J = pp_.tile([128, 128], f32)
            ones = pp_.tile([128, 128], f32)
            blog = pp_.tile([128, NT, DO], f32)

            # uT+Wp gate mm1 of iteration 0 -> load first, on separate queues.
            # u_nat/J are not needed until mm2 (~25us in); emitted later so
            # the serialized DMA engines finish uT+Wp first.
            nc.scalar.dma_start(Wp[:, 0:3, :], Wp_d[:, 0:3, :])
            nc.sync.dma_start(uT[:, :, 0:128], uT_d[:, :, 0:128])
            nc.scalar.dma_start(Wp[:, 3:6, :], Wp_d[:, 3:6, :])
            nc.scalar.dma_start(Wp[:, 6:9, :], Wp_d[:, 6:9, :])
            nc.sync.dma_start(uT[:, :, 128:256], uT_d[:, :, 128:256])
            nc.gpsimd.memset(ones[:], 1.0)
            # Warm the PE HAM clock while the uT/Wp DMAs are in flight so
            # iteration 0's mm1 runs at 2.4 GHz instead of 1.2 GHz.
            if prewarm:
                pw_ps = ps_t.tile([128, 96], f32, name="pw", tag="wm")
                for k in range(prewarm):
                    nc.tensor.matmul(
                        pw_ps[:], ones[:, 0:128], ones[:, 0:96],
                        start=True, stop=True,
                    )
            deferred_loads = [False]

            def _emit_deferred_loads(anchor):
                # u_nat/J are only needed by mm2 (~25us in).  The scheduler
                # ignores emission order for dep-free instructions, so gate
                # these loads on an iteration-0 instruction to keep the DMA
                # engines free for the uT/Wp loads that gate mm1.
                if deferred_loads[0]:
                    return
                deferred_loads[0] = True
                for h in range(NB):
                    d = nc.gpsimd.dma_start(u_nat[:, h, :], u_nat_d[h])
                    add_dep_helper(d.ins, anchor.ins, sync=True,
                                   reason="defer u_nat load past uT/Wp")
                dj = nc.gpsimd.dma_start(J[:], Jm_d[:])
                add_dep_helper(dj.ins, anchor.ins, sync=True,
                               reason="defer J load past uT/Wp")

            for it in range(N_ITER * reps):
                rep, it = divmod(it, N_ITER)
                last = it == N_ITER - 1
                if it == 0:
                    # b==0 => c uniform: feed W directly, fold 1/(10*16)
                    # into the squash constants (s_dev = 10 * s_true).
                    CW = Wp
                    A2 = 100.0
                else:
                    # ---- softmax over d (expanded layout), fold 1/16 ----
                    # Per-t chains so CW_t becomes ready while the PE is
                    # still working through this iteration's J-matmuls.
                    e = ip_.tile([128, NT, DO], f32, name=f"e{rep}_{it}", tag="e")
                    den16 = ip_.tile([128, NT], f32, name=f"den{rep}_{it}", tag="den")
                    recip16 = ip_.tile([128, NT], f32, name=f"rc{rep}_{it}", tag="rc")
                    CW = ip_.tile([128, NT, DO], f32, name=f"cw{rep}_{it}", tag="cw")
                    A2 = 1.0 / 256.0
                    for t in range(NT):
                        nc.scalar.activation(
                            e[:, t, :], blog[:, t, :], AF.Exp,
                            accum_out=den16[:, t : t + 1],
                        )
                        nc.vector.reciprocal(
                            recip16[:, t : t + 1], den16[:, t : t + 1]
                        )
                        # CW = (e * recip16) * Wp   == (c/16) * W
                        nc.vector.scalar_tensor_tensor(
                            CW[:, t, :], e[:, t, :], recip16[:, t : t + 1],
                            Wp[:, t, :], op0=ALU.mult, op1=ALU.mult,
                        )
                # ---- mm1: s_dev[b,(d,o)] = sum_(r,i) uT.T @ CW ----
                # h-outer so half 0's PSUM drain + bounce DMA overlap half 1.
                s_sb = ip_.tile([128, NB, DO], f32, name=f"s{rep}_{it}", tag="s")
                inb = dp_.tile([NB, 128, DO], f32, name=f"inb{rep}_{it}", tag="inb")
                outb = dp_.tile([NB, 128, DO], f32, name=f"outb{rep}_{it}", tag="outb")
                copy0 = None
                for h in range(NB):
                    s_ps = ps_s.tile(
                        [128, DO], f32, name=f"sps{rep}_{it}_{h}", tag=f"sps{h}"
                    )
                    for t in range(NT):
                        nc.tensor.matmul(
                            s_ps[:],
                            uT[:, t, h * 128 : (h + 1) * 128],
                            CW[:, t, :],
                            start=(t == 0),
                            stop=(t == NT - 1),
                        )
                    cp = nc.vector.tensor_copy(s_sb[:, h, :], s_ps[:])
                    if h == 0:
                        copy0 = cp
                    # all AR-path DMAs stay on the sync engine: a dma_start's
                    # sem wait blocks the issuing engine's in-order sequencer,
                    # and SP has no compute to stall.
                    nc.sync.dma_start(inb[h], s_sb[:, h, :])
                # ---- AllReduce partial s over the 8 cores ----
                nc.gpsimd.collective_compute(
                    "AllReduce", ALU.add, replica_groups=rg,
                    ins=[inb.opt()], outs=[outb.opt()],
                )
                if it == 0 and rep == 0:
                    _emit_deferred_loads(copy0)
                # keep the PE array's HAM clock warm through the collective:
                # a chain of tiny matmuls gated on s_sb (i.e. after mm1).
                if warm_mms:
                    wm_ps = ps_t.tile(
                        [128, 96], f32, name=f"wm{rep}_{it}", tag="wm"
                    )
                    for k in range(warm_mms):
                        wmi = nc.tensor.matmul(
                            wm_ps[:],
                            ones[:, 0:128],
                            ones[:, 0:96],
                            start=True,
                            stop=True,
                        )
                        if k == 0:
                            add_dep_helper(
                                wmi.ins, copy0.ins, sync=True,
                                reason="warm chain starts after mm1 drain",
                            )
                sf = ip_.tile([128, NB, DO], f32, name=f"sf{rep}_{it}", tag="sf")
                nc.sync.dma_start(
                    sf[:], outb[:].rearrange("h p f -> p h f")
                )
                # ---- squash with global norm over the full batch ----
                # s_dev = A*s_true  =>  v = s_dev * sqrt(T)/(A^2 + T),
                # T = sum(s_dev^2).
                sqscr = sp_.tile([128, NB * DO], f32, name=f"sq{rep}_{it}", tag="sq")
                ppsum = sp_.tile([128, 1], f32, name=f"pps{rep}_{it}", tag="pps")
                nc.scalar.activation(
                    sqscr[:], sf[:].rearrange("p h f -> p (h f)"), AF.Square,
                    accum_out=ppsum[:],
                )
                # T broadcast to every partition via ones-matmul
                T_ps = ps_t.tile([128, 1], f32, name=f"T{rep}_{it}", tag="T")
                nc.tensor.matmul(T_ps[:], ones[:], ppsum[:], start=True, stop=True)
                q = sp_.tile([128, 1], f32, name=f"q{rep}_{it}", tag="q")
                nc.vector.tensor_scalar_add(q[:], T_ps[:], A2)
                qinv = sp_.tile([128, 1], f32, name=f"qi{rep}_{it}", tag="qi")
                nc.vector.reciprocal(qinv[:], q[:])
                rt = sp_.tile([128, 1], f32, name=f"rt{rep}_{it}", tag="rt")
                nc.scalar.activation(rt[:], T_ps[:], AF.Sqrt)
                g = sp_.tile([128, 1], f32, name=f"g{rep}_{it}", tag="g")
                nc.vector.tensor_tensor(g[:], rt[:], qinv[:], op=ALU.mult)
                if last:
                    v_sb = ip_.tile([128, NB, DO], f32, name=f"v{rep}_{it}", tag="v")
                    for h in range(NB):
                        nc.vector.tensor_scalar_mul(
                            v_sb[:, h, :], sf[:, h, :], g[:, 0:1]
                        )
                    for h in range(NB):
                        nc.sync.dma_start(v_out_d[h], v_sb[:, h, :])
                else:
                    # ---- mm2 on sf directly (G = g*(u.T@sf)); the squash
                    # scalar g folds into the H multiply, so mm2 starts
                    # right at the AR return, before the squash chain ----
                    Hred = ip_.tile([128, NT, D], f32, name=f"hr{rep}_{it}", tag="hr")
                    for t in range(NT):
                        G_ps = ps_g.tile([128, DO], f32, name=f"G{rep}_{it}_{t}", tag="G")
                        for h in range(NB):
                            nc.tensor.matmul(
                                G_ps[:],
                                u_nat[:, h, t * 128 : (t + 1) * 128],
                                sf[:, h, :],
                                start=(h == 0),
                                stop=(h == NB - 1),
                            )
                        # Hred[:,t,d] = sum_o Wp * G * g   (g: per-partition scalar)
                        Ht = sp_.tile([128, DO], f32, name=f"ht{rep}_{it}_{t}", tag="ht")
                        nc.vector.scalar_tensor_tensor(
                            Ht[:], G_ps[:], g[:, 0:1], Wp[:, t, :],
                            op0=ALU.mult, op1=ALU.mult,
                        )
                        nc.vector.reduce_sum(
                            Hred[:, t, :],
                            Ht[:].rearrange("p (d o) -> p d o", d=D, o=O),
                            axis=mybir.AxisListType.X,
                        )
                    # ---- i-sum + broadcast via J; blog update ----
                    for t in range(NT):
                        bd_ps = ps_bd.tile([128, D], f32, name=f"bd{rep}_{it}_{t}", tag="bd")
                        nc.tensor.matmul(
                            bd_ps[:], J[:], Hred[:, t, :], start=True, stop=True
                        )
                        blog_v = blog[:, t, :].rearrange("p (d o) -> p d o", d=D, o=O)
                        bd_bc = bd_ps[:].unsqueeze(2).broadcast_to([128, D, O])
                        if it == 0:
                            # blog starts at 0: first update is a plain copy
                            nc.vector.tensor_copy(blog_v, bd_bc)
                        else:
                            nc.vector.tensor_tensor(
                                blog_v, blog_v, bd_bc, op=ALU.add
                            )

    _split_excess_waits(nc, 1)
    return nc


_NC_CACHE = {}


def _get_nc(reps=1, warm_mms=50):
    key = (reps, warm_mms)
    if key not in _NC_CACHE:
        _NC_CACHE[key] = _build_nc(reps=reps, warm_mms=warm_mms)
    return _NC_CACHE[key]


def _prep_core_inputs(u, W, c):
    r0, r1 = c * RL, (c + 1) * RL
    u2 = np.ascontiguousarray(u[:, r0:r1, :]).reshape(B, KRI)
    u_nat = np.ascontiguousarray(u2.reshape(NB, 128, KRI))
    uT = np.ascontiguousarray(
        np.ascontiguousarray(u2.T).reshape(NT, 128, B).transpose(1, 0, 2)
    )
    Wp2 = np.ascontiguousarray(W[0, r0:r1].transpose(0, 3, 1, 2)).reshape(KRI, DO)
    Wp = np.ascontiguousarray(Wp2.reshape(NT, 128, DO).transpose(1, 0, 2))
    return {"u_nat": u_nat, "uT": uT, "Wp": Wp}


def kernel(u, W, _trace=False, _reps=1, _warm_mms=50):
    u = np.asarray(u, dtype=np.float32)
    W = np.asarray(W, dtype=np.float32)
    assert u.shape == (B, R, I_CH) and W.shape == (1, R, D, O, I_CH)
    Jm = np.kron(np.eye(16, dtype=np.float32), np.ones((8, 8), np.float32))
    in_maps = []
    for c in range(N_CORES):
        m = _prep_core_inputs(u, W, c)
        m["Jm"] = Jm
        in_maps.append(m)
    nc = _get_nc(_reps, _warm_mms)
    res = run_bass_kernel_spmd(
        nc, in_maps, core_ids=list(range(N_CORES)), trace=_trace
    )
    v = res.results[0]["v_out"].reshape(B, D, O).astype(np.float32)
    if _trace:
        return v, res
    return v
